# revision 12
# baseline (speedup 1.0000x reference)
"""Trainium2 Bass kernel: multi-head causal self-attention with RoPE.

Computes, for x:[B,S,D], Wq/Wk/Wv/Wo:[D,D] (B=2, S=2048, D=1024, H=16 heads,
hd=64):
    q/k/v = (x @ W{q,k,v}.T) -> [B,H,S,hd];  q,k = rope(q), rope(k)
    out   = softmax(causal(q k^T / sqrt(hd))) v   -> merge heads -> @ Wo.T

Sharding: 8 NeuronCores = (2 batches) x (4 head-groups of 4 heads).  Each
core computes its 4 heads' attention plus the partial output projection
(columns of Wo belonging to its heads); the host sums the 4 partial outputs
per batch.

Per-core dataflow (everything in "transposed" space so no PE transposes are
needed):
    xT [D,S] -> QT,KT [hd,S] per head (fp32r matmuls) -> RoPE (DVE shuffle
    + mul/add) -> scoresT[k,q] = KT^T-slice matmuls -> exp on ACT (no
    max-subtraction: |scores/8| <= ~3.2) -> PV with a ones-column appended
    to V so row 64 of the accumulator is the softmax denominator ->
    normalize -> output projection from the transposed head outputs.
"""

import sys

sys.path.insert(0, "/opt/trn_rl_repo")

import numpy as np

import concourse.bass as bass
import concourse.mybir as mybir
import concourse.tile as tile
from concourse.bass_utils import run_bass_kernel_spmd

F32 = mybir.dt.float32
F32R = mybir.dt.float32r
AF = mybir.ActivationFunctionType
OP = mybir.AluOpType

# stream_shuffle's 32-entry mask is a per-quadrant partition permutation
# (applied identically to all four 32-partition quadrants).  We therefore
# store head dims interleaved -- partition 64h+2i holds dim i, 64h+2i+1
# holds dim 32+i -- so the RoPE pair swap is an adjacent-pair exchange.
# The interleave is a shared permutation of Q and K dims (folded into the
# weight slices and rope tables on the host), which leaves q.k scores
# unchanged.
SWAP_MASK = [i ^ 1 for i in range(32)]

HD = 64
HALF = HD // 2
ROPE_BASE = 10000.0


def _split_waits(nc, maxw=1):
    """walrus in this container rejects instructions with more than a couple
    of semaphore waits; hoist excess waits onto preceding NoOps."""
    ctr = 0
    for bb in nc.main_func.blocks:
        insts = bb.instructions
        new = []
        changed = False
        for ins in insts:
            si = ins.sync_info
            if si is not None and si.on_wait and len(si.on_wait) > maxw:
                waits = list(si.on_wait)
                keep, rest = waits[:maxw], waits[maxw:]
                for i in range(0, len(rest), maxw):
                    ctr += 1
                    new.append(mybir.InstNoOp(
                        name=f"WSPLIT-{ctr}", opcode="NoOp", engine=ins.engine,
                        sync_info=mybir.SyncInfo(on_wait=rest[i:i + maxw], on_update=[])))
                si.on_wait = keep
                changed = True
            new.append(ins)
        if changed:
            bb.instructions = new


def build_program(S, D, HPC=4, repeat=1):
    """One-core SPMD program: attention for HPC heads of one batch."""
    NKT = D // 128          # k-tiles over the embedding dim
    NSC = S // 512          # 512-wide s-chunks
    NST = S // 128          # 128-wide s-tiles
    G = HPC // 2            # head pairs
    E = HPC * HD            # per-core head dims

    nc = bass.Bass()
    xT = nc.declare_dram_parameter("xT", [D, S], F32R, isOutput=False)
    wq = nc.declare_dram_parameter("wq", [D, E], F32R, isOutput=False)
    wk = nc.declare_dram_parameter("wk", [D, E], F32R, isOutput=False)
    wv = nc.declare_dram_parameter("wv", [D, E], F32R, isOutput=False)
    wo = nc.declare_dram_parameter("wo", [E, D], F32R, isOutput=False)
    cs = nc.declare_dram_parameter("cs", [128, S], F32, isOutput=False)
    sn = nc.declare_dram_parameter("sn", [128, S], F32, isOutput=False)
    tri = nc.declare_dram_parameter("tri", [128, 128], F32R, isOutputFalse := False)
    out = nc.declare_dram_parameter("out", [S, D], F32, isOutput=True)

    with tile.TileContext(nc) as tc, \
         nc.allow_low_precision(reason="float32r operands feed the PE at full rate"):
        with (tc.tile_pool(name="wp", bufs=1) as wp,
              tc.tile_pool(name="xp", bufs=2) as xp,
              tc.tile_pool(name="rt", bufs=3) as rt,
              tc.tile_pool(name="ptp", bufs=4) as ptp,
              tc.tile_pool(name="rcp", bufs=2) as rcp,
              tc.tile_pool(name="oevp", bufs=2) as oevp,
              tc.tile_pool(name="qkv_ps", bufs=3, space="PSUM") as qkv_ps,
              tc.tile_pool(name="mm_ps", bufs=3, space="PSUM") as mm_ps,
              tc.tile_pool(name="ot_ps", bufs=2, space="PSUM") as ot_ps):

            # ---- persistent tiles
            wq_s = wp.tile([128, NKT * E], F32R, name="wq_s")
            wk_s = wp.tile([128, NKT * E], F32R, name="wk_s")
            wv_s = wp.tile([128, NKT * E], F32R, name="wv_s")
            wo_s = wp.tile([128, G * D], F32R, name="wo_s")
            cs_s = wp.tile([128, S], F32, name="cs_s")
            sn_s = wp.tile([128, S], F32, name="sn_s")
            tri_s = wp.tile([128, 128], F32R, name="tri_s")
            on_s = wp.tile([128, 64], F32R, name="on_s")
            qtr = wp.tile([128, G * S], F32R, name="qtr")
            ktr = wp.tile([128, G * S], F32R, name="ktr")
            vv = wp.tile([128, NST * HPC * (HD + 1)], F32R, name="vv")
            otn = wp.tile([128, G * S], F32R, name="otn")

            for kt in range(NKT):
                nc.sync.dma_start(wq_s[:, kt * E:(kt + 1) * E], wq[kt * 128:(kt + 1) * 128, :])
                nc.sync.dma_start(wk_s[:, kt * E:(kt + 1) * E], wk[kt * 128:(kt + 1) * 128, :])
                nc.sync.dma_start(wv_s[:, kt * E:(kt + 1) * E], wv[kt * 128:(kt + 1) * 128, :])
            for g in range(G):
                nc.sync.dma_start(wo_s[:, g * D:(g + 1) * D], wo[g * 128:(g + 1) * 128, :])
            nc.sync.dma_start(cs_s[:], cs[:])
            nc.sync.dma_start(sn_s[:], sn[:])
            nc.sync.dma_start(tri_s[:], tri[:])
            # ones columns of V+ (for the softmax denominator)
            vv_r = vv[:].rearrange("p (st h c) -> p st h c", st=NST, h=HPC, c=HD + 1)
            nc.vector.memset(vv_r[:, :, :, HD:HD + 1].bitcast(F32), 1.0)
            nc.vector.memset(on_s[:].bitcast(F32), 1.0)

            for rep in range(repeat):
                # ================= QKV + RoPE =================
                for sc in range(NSC):
                    xts = []
                    for kt in range(NKT):
                        xt = xp.tile([128, 512], F32R, tag=f"x{kt}", name=f"x{kt}")
                        nc.sync.dma_start(xt[:], xT[kt * 128:(kt + 1) * 128, sc * 512:(sc + 1) * 512])
                        xts.append(xt)

                    # Q^T and K^T, one head-pair (128 dims) at a time
                    for w_s, dst in ((wq_s, qtr), (wk_s, ktr)):
                        for g in range(G):
                            ps = qkv_ps.tile([128, 512], F32, tag="qkv", name="ps")
                            for kt in range(NKT):
                                nc.tensor.matmul(
                                    ps[:], w_s[:, kt * E + g * 128: kt * E + (g + 1) * 128],
                                    xts[kt][:], start=(kt == 0), stop=(kt == NKT - 1))
                            # RoPE: rot = ps*cos + swap(ps)*sgn_sin
                            qsw = rt.tile([128, 512], F32, tag="qsw", name="qsw")
                            m1 = rt.tile([128, 512], F32, tag="m1", name="m1")
                            m2 = rt.tile([128, 512], F32, tag="m2", name="m2")
                            nc.vector.stream_shuffle(qsw[:], ps[:], SWAP_MASK)
                            nc.vector.tensor_tensor(m1[:], ps[:], cs_s[:, sc * 512:(sc + 1) * 512], OP.mult)
                            nc.gpsimd.tensor_tensor(m2[:], qsw[:], sn_s[:, sc * 512:(sc + 1) * 512], OP.mult)
                            nc.vector.tensor_tensor(
                                dst[:, g * S + sc * 512: g * S + (sc + 1) * 512], m1[:], m2[:], OP.add)

                    # V (natural layout) for the 4 s-tiles of this chunk
                    for stl in range(4):
                        st = sc * 4 + stl
                        ps = qkv_ps.tile([128, 512], F32, tag="qkv", name="psv")
                        for kt in range(NKT):
                            nc.tensor.matmul(
                                ps[:, 0:E], xts[kt][:, stl * 128:(stl + 1) * 128],
                                wv_s[:, kt * E:(kt + 1) * E], start=(kt == 0), stop=(kt == NKT - 1))
                        dst = vv_r[:, st, :, 0:HD]
                        nc.scalar.copy(dst, ps[:, 0:E].rearrange("p (h c) -> p h c", h=HPC, c=HD))

                # ================= attention + output projection =================
                for qc in range(NSC):
                    for h in range(HPC):
                        g, hp = h // 2, (h % 2) * 64
                        ot = ot_ps.tile([128, 512], F32, tag="ot", name="ot")
                        nkt_q = 4 * qc + 4
                        for kt in range(nkt_q):
                            coff = max(0, 128 * kt - 512 * qc)
                            sct = mm_ps.tile([128, 512], F32, tag="sc", name="sct")
                            nc.tensor.matmul(
                                sct[:, coff:512],
                                ktr[hp:hp + 64, g * S + kt * 128: g * S + (kt + 1) * 128],
                                qtr[hp:hp + 64, g * S + qc * 512 + coff: g * S + (qc + 1) * 512],
                                start=True, stop=True)
                            pt = ptp.tile([128, 512], F32R, tag="pt", name="pt")
                            nc.scalar.activation(pt[:, coff:512], sct[:, coff:512], AF.Exp, scale=0.125)
                            if kt >= 4 * qc:  # tile containing the diagonal
                                nc.gpsimd.tensor_tensor(
                                    pt[:, coff:coff + 128], pt[:, coff:coff + 128], tri_s[:], OP.mult)
                            nc.tensor.matmul(
                                ot[0:HD + 1, coff:512],
                                vv[:, kt * HPC * (HD + 1) + h * (HD + 1): kt * HPC * (HD + 1) + (h + 1) * (HD + 1)],
                                pt[:, coff:512], start=(kt == 0), stop=(kt == nkt_q - 1))
                        # normalize by the denominator row and store transposed:
                        # reciprocal of row 64, broadcast across 64 partitions
                        # via a K=1 matmul, then multiply out of PSUM.
                        rc = rcp.tile([128, 512], F32R, tag="rc", name="rc")
                        nc.vector.reciprocal(rc[HD:HD + 1, :], ot[HD:HD + 1, :])
                        rcb = mm_ps.tile([128, 512], F32, tag="sc", name="rcb")
                        nc.tensor.matmul(rcb[0:HD, :], on_s[HD:HD + 1, 0:HD],
                                         rc[HD:HD + 1, :], start=True, stop=True)
                        nc.scalar.copy(rc[0:HD, :], rcb[0:HD, :])
                        nc.vector.tensor_tensor(
                            otn[hp:hp + 64, g * S + qc * 512: g * S + (qc + 1) * 512],
                            ot[0:HD, :], rc[0:HD, :], OP.mult)

                    # output projection for the 4 s-tiles of this q-chunk
                    CW = min(512, D)
                    for stl in range(4):
                        st = qc * 4 + stl
                        oev = oevp.tile([128, D], F32, tag="oev", name="oev")
                        for nch in range(D // CW):
                            op = mm_ps.tile([128, 512], F32, tag="sc", name="opps")
                            for g in range(G):
                                nc.tensor.matmul(
                                    op[:, 0:CW], otn[:, g * S + st * 128:g * S + (st + 1) * 128],
                                    wo_s[:, g * D + nch * CW: g * D + (nch + 1) * CW],
                                    start=(g == 0), stop=(g == G - 1))
                            if nch % 2 == 0:
                                nc.scalar.copy(oev[:, nch * CW:(nch + 1) * CW], op[:, 0:CW])
                            else:
                                nc.vector.tensor_copy(oev[:, nch * CW:(nch + 1) * CW], op[:, 0:CW])
                        nc.sync.dma_start(out[st * 128:(st + 1) * 128, :], oev[:])

    _split_waits(nc)
    return nc


def _rope_tables(S):
    # interleaved dim order: within each 64-partition head block, partition
    # j=2i holds dim i (gets cos, -sin), j=2i+1 holds dim 32+i (cos, +sin)
    inv = 1.0 / (ROPE_BASE ** (np.arange(HALF, dtype=np.float64) / HALF))
    ang = np.arange(S, dtype=np.float64)[:, None] * inv[None, :]  # [S, HALF]
    cos, sin = np.cos(ang), np.sin(ang)
    j = np.arange(128) % HD
    freq = j // 2
    cs = cos[:, freq].T.astype(np.float32)                # [128, S]
    sgn = np.where(j % 2 == 0, -1.0, 1.0)
    sn = (sin[:, freq] * sgn[None, :]).T.astype(np.float32)
    return np.ascontiguousarray(cs), np.ascontiguousarray(sn)


def _interleave_perm(n_heads):
    """Permutation of head-dim rows: new row 64h+2i <- old 64h+i,
    new row 64h+2i+1 <- old 64h+32+i."""
    perm = np.empty(n_heads * HD, dtype=np.int64)
    for h in range(n_heads):
        base = h * HD
        for i in range(HALF):
            perm[base + 2 * i] = base + i
            perm[base + 2 * i + 1] = base + HALF + i
    return perm


def kernel(x, Wq, Wk, Wv, Wo):
    B, S, D = x.shape
    H = 16
    HPC = 4                      # heads per core
    GROUPS = H // HPC            # 4 head-groups
    N_CORES = B * GROUPS

    x = np.asarray(x, dtype=np.float32)
    Wq, Wk, Wv, Wo = (np.asarray(w, dtype=np.float32) for w in (Wq, Wk, Wv, Wo))

    cs, sn = _rope_tables(S)
    tri = np.triu(np.ones((128, 128), dtype=np.float32))  # keep k<=q in [k,q] layout
    xTs = [np.ascontiguousarray(x[b].T) for b in range(B)]

    perm = _interleave_perm(HPC)
    in_maps = []
    for c in range(N_CORES):
        b, hg = divmod(c, GROUPS)
        e0 = hg * HPC * HD
        e1 = e0 + HPC * HD
        in_maps.append({
            "xT": xTs[b],
            "wq": np.ascontiguousarray(Wq[e0:e1, :][perm].T),
            "wk": np.ascontiguousarray(Wk[e0:e1, :][perm].T),
            "wv": np.ascontiguousarray(Wv[e0:e1, :].T),
            "wo": np.ascontiguousarray(Wo[:, e0:e1].T),
            "cs": cs, "sn": sn, "tri": tri,
        })

    nc = build_program(S, D, HPC)
    res = run_bass_kernel_spmd(nc, in_maps, list(range(N_CORES)))

    out = np.zeros((B, S, D), dtype=np.float64)
    for c in range(N_CORES):
        b = c // GROUPS
        out[b] += res.results[c]["out"].astype(np.float64)
    return out.astype(np.float32)


if __name__ == "__main__":
    # mini self-test: one core, small S/D, against a numpy model
    S, D, HPC = 512, 256, 4
    rng = np.random.default_rng(0)
    x = rng.standard_normal((S, D)).astype(np.float32)
    bound = 1.0 / np.sqrt(D)
    Wq, Wk, Wv = (rng.uniform(-bound, bound, (HPC * HD, D)).astype(np.float32) for _ in range(3))
    Wo = rng.uniform(-bound, bound, (D, HPC * HD)).astype(np.float32)

    # numpy reference (same math as reference.py, restricted to HPC heads)
    q = (x @ Wq.T).reshape(S, HPC, HD).transpose(1, 0, 2)
    k = (x @ Wk.T).reshape(S, HPC, HD).transpose(1, 0, 2)
    v = (x @ Wv.T).reshape(S, HPC, HD).transpose(1, 0, 2)
    inv = 1.0 / (ROPE_BASE ** (np.arange(HALF) / HALF))
    ang = np.arange(S)[:, None] * inv[None, :]
    cosr, sinr = np.cos(ang), np.sin(ang)

    def rope(t):
        t1, t2 = t[..., :HALF], t[..., HALF:]
        return np.concatenate([t1 * cosr - t2 * sinr, t1 * sinr + t2 * cosr], -1)

    q, k = rope(q), rope(k)
    sc_ = np.einsum("hqd,hkd->hqk", q, k) / np.sqrt(HD)
    mask = np.tril(np.ones((S, S), dtype=bool))
    sc_ = np.where(mask, sc_, -np.inf)
    p = np.exp(sc_ - sc_.max(-1, keepdims=True))
    p /= p.sum(-1, keepdims=True)
    ref = np.einsum("hqk,hkd->hqd", p, v).transpose(1, 0, 2).reshape(S, HPC * HD) @ Wo.T

    cs, sn = _rope_tables(S)
    tri = np.triu(np.ones((128, 128), dtype=np.float32))
    perm = _interleave_perm(HPC)
    in_map = {
        "xT": np.ascontiguousarray(x.T),
        "wq": np.ascontiguousarray(Wq[perm].T),
        "wk": np.ascontiguousarray(Wk[perm].T),
        "wv": np.ascontiguousarray(Wv.T),
        "wo": np.ascontiguousarray(Wo.T),
        "cs": cs, "sn": sn, "tri": tri,
    }
    nc = build_program(S, D, HPC)
    res = run_bass_kernel_spmd(nc, [in_map], [0])
    got = res.results[0]["out"]
    err = np.abs(got - ref)
    rel = err.max() / np.abs(ref).max()
    rms = np.sqrt((err ** 2).mean()) / np.sqrt((ref ** 2).mean())
    print(f"mini: max abs err {err.max():.3e}  max rel {rel:.3e}  rms rel {rms:.3e}")


# revision 43
# speedup vs baseline: 824.1301x; 824.1301x over previous
"""Trainium2 Bass kernel: multi-head causal self-attention with RoPE.

Computes, for x:[B,S,D], Wq/Wk/Wv/Wo:[D,D] (B=2, S=2048, D=1024, H=16 heads,
hd=64):
    q/k/v = (x @ W{q,k,v}.T) -> [B,H,S,hd];  q,k = rope(q), rope(k)
    out   = softmax(causal(q k^T / sqrt(hd))) v   -> merge heads -> @ Wo.T

Sharding: 8 NeuronCores = (2 batches) x (4 head-groups of 4 heads).  Each
core computes its 4 heads' attention plus the partial output projection
(columns of Wo belonging to its heads); the host sums the 4 partial outputs
per batch.

Per-core dataflow (everything in "transposed" space so no PE transposes are
needed):
    xT [D,S] -> QT,KT [hd,S] per head (fp32r matmuls) -> RoPE (DVE shuffle
    + mul/add) -> scoresT[k,q] = KT^T-slice matmuls -> exp on ACT (no
    max-subtraction: |scores/8| <= ~3.2) -> PV with a ones-column appended
    to V so row 64 of the accumulator is the softmax denominator ->
    normalize -> output projection from the transposed head outputs.
"""

import sys

sys.path.insert(0, "/opt/trn_rl_repo")

import numpy as np

import concourse.bass as bass
import concourse.mybir as mybir
import concourse.tile as tile
from concourse.bass_utils import run_bass_kernel_spmd

F32 = mybir.dt.float32
F32R = mybir.dt.float32r
AF = mybir.ActivationFunctionType
OP = mybir.AluOpType

# stream_shuffle's 32-entry mask is a per-quadrant partition permutation
# (applied identically to all four 32-partition quadrants).  We therefore
# store head dims interleaved -- partition 64h+2i holds dim i, 64h+2i+1
# holds dim 32+i -- so the RoPE pair swap is an adjacent-pair exchange.
# The interleave is a shared permutation of Q and K dims (folded into the
# weight slices and rope tables on the host), which leaves q.k scores
# unchanged.
SWAP_MASK = [i ^ 1 for i in range(32)]

HD = 64
HALF = HD // 2
ROPE_BASE = 10000.0


def _split_waits(nc, maxw=1):
    """walrus in this container rejects instructions with more than a couple
    of semaphore waits; hoist excess waits onto preceding NoOps."""
    ctr = 0
    for bb in nc.main_func.blocks:
        insts = bb.instructions
        new = []
        changed = False
        for ins in insts:
            si = ins.sync_info
            if si is not None and si.on_wait and len(si.on_wait) > maxw:
                waits = list(si.on_wait)
                keep, rest = waits[:maxw], waits[maxw:]
                for i in range(0, len(rest), maxw):
                    ctr += 1
                    new.append(mybir.InstNoOp(
                        name=f"WSPLIT-{ctr}", opcode="NoOp", engine=ins.engine,
                        sync_info=mybir.SyncInfo(on_wait=rest[i:i + maxw], on_update=[])))
                si.on_wait = keep
                changed = True
            new.append(ins)
        if changed:
            bb.instructions = new


def build_program(S, D, HPC=4, repeat=1, use_loop=False, phase=4):
    """phase: 1 = QKV+rope only, 2 = +scores/exp/PV, 3 = full kernel."""
    """One-core SPMD program: attention for HPC heads of one batch."""
    NKT = D // 128          # k-tiles over the embedding dim
    NSC = S // 512          # 512-wide s-chunks
    NST = S // 128          # 128-wide s-tiles
    G = HPC // 2            # head pairs
    E = HPC * HD            # per-core head dims

    nc = bass.Bass()
    # pre-tiled host layouts: xT[p, sc*NKT*512 + kt*512 + s'] = x[sc*512+s', kt*128+p]
    xT = nc.declare_dram_parameter("xT", [128, S * NKT], F32R, isOutput=False)
    wq = nc.declare_dram_parameter("wq", [128, NKT * E], F32R, isOutput=False)
    wk = nc.declare_dram_parameter("wk", [128, NKT * E], F32R, isOutput=False)
    wv = nc.declare_dram_parameter("wv", [128, NKT * E], F32R, isOutput=False)
    wo = nc.declare_dram_parameter("wo", [128, G * D], F32R, isOutput=False)
    cs = nc.declare_dram_parameter("cs", [128, S], F32, isOutput=False)
    sn = nc.declare_dram_parameter("sn", [128, S], F32, isOutput=False)
    tri = nc.declare_dram_parameter("tri", [128, 128], F32R, isOutput=False)
    idn = nc.declare_dram_parameter("idn", [128, 128], F32R, isOutput=False)
    out = nc.declare_dram_parameter("out", [S, D], F32, isOutput=True)

    with tile.TileContext(nc) as tc, \
         nc.allow_low_precision(reason="float32r operands feed the PE at full rate"):
        with (tc.tile_pool(name="wp", bufs=1) as wp,
              tc.tile_pool(name="xp", bufs=2) as xp,
              tc.tile_pool(name="rt", bufs=3) as rt,
              tc.tile_pool(name="ptp", bufs=4) as ptp,
              tc.tile_pool(name="rcp", bufs=3) as rcp,
              tc.tile_pool(name="oevp", bufs=2) as oevp,
              tc.tile_pool(name="ps", bufs=1, space="PSUM") as ps_pool,
              tc.tile_pool(name="ot_ps", bufs=2, space="PSUM") as ot_ps):
            qkv_ps = mm_ps = ps_pool  # shared PSUM pool; tags set per tile

            # ---- persistent tiles
            wq_s = wp.tile([128, NKT * E], F32R, name="wq_s")
            wk_s = wp.tile([128, NKT * E], F32R, name="wk_s")
            wv_s = wp.tile([128, NKT * E], F32R, name="wv_s")
            wo_s = wp.tile([128, G * D], F32R, name="wo_s")
            cs_s = wp.tile([128, S], F32, name="cs_s")
            sn_s = wp.tile([128, S], F32, name="sn_s")
            tri_s = wp.tile([128, 128], F32R, name="tri_s")
            on_s = wp.tile([128, 64], F32R, name="on_s")
            idn_s = wp.tile([128, 128], F32R, name="idn_s")
            qtr = wp.tile([128, G * S], F32R, name="qtr")
            ktr = wp.tile([128, G * S], F32R, name="ktr")
            vv = wp.tile([128, NST * HPC * (HD + 1)], F32R, name="vv")
            otn = wp.tile([128, G * S], F32R, name="otn")

            nc.sync.dma_start(wq_s[:], wq[:])
            nc.sync.dma_start(wk_s[:], wk[:])
            nc.sync.dma_start(wv_s[:], wv[:])
            nc.sync.dma_start(wo_s[:], wo[:])
            nc.sync.dma_start(cs_s[:], cs[:])
            nc.sync.dma_start(sn_s[:], sn[:])
            nc.sync.dma_start(tri_s[:], tri[:])
            nc.sync.dma_start(idn_s[:], idn[:])
            # ones columns of V+ (for the softmax denominator)
            vv_r = vv[:].rearrange("p (st h c) -> p st h c", st=NST, h=HPC, c=HD + 1)
            nc.vector.memset(vv_r[:, :, :, HD:HD + 1].bitcast(F32), 1.0)
            nc.vector.memset(on_s[:].bitcast(F32), 1.0)

            def body():
                # ================= QKV + RoPE =================
                def qkv_chunk(sc):
                    xta = xp.tile([128, NKT * 512], F32R, tag="x", name="xta")
                    nc.sync.dma_start(xta[:], xT[:, sc * NKT * 512:(sc + 1) * NKT * 512])
                    xts = [xta[:, kt * 512:(kt + 1) * 512] for kt in range(NKT)]

                    if phase == -1:
                        return
                    # Q^T and K^T, one head-pair (128 dims) at a time
                    for w_s, dst in ((wq_s, qtr), (wk_s, ktr)):
                        for g in range(G):
                            ps = qkv_ps.tile([128, 512], F32, tag="mm", bufs=2, name="ps")
                            for kt in range(NKT):
                                nc.tensor.matmul(
                                    ps[:], w_s[:, kt * E + g * 128: kt * E + (g + 1) * 128],
                                    xts[kt], start=(kt == 0), stop=(kt == NKT - 1))
                            if phase == 0:
                                continue
                            # RoPE: rot = ps*cos + swap(ps)*sgn_sin
                            qsw = rt.tile([128, 512], F32, tag="qsw", name="qsw")
                            m1 = rt.tile([128, 512], F32, tag="m1", name="m1")
                            m2 = rt.tile([128, 512], F32, tag="m2", name="m2")
                            nc.vector.stream_shuffle(qsw[:], ps[:], SWAP_MASK)
                            nc.vector.tensor_tensor(m1[:], ps[:], cs_s[:, sc * 512:(sc + 1) * 512], OP.mult)
                            nc.gpsimd.tensor_tensor(m2[:], qsw[:], sn_s[:, sc * 512:(sc + 1) * 512], OP.mult)
                            nc.vector.tensor_tensor(
                                dst[:, g * S + sc * 512: g * S + (sc + 1) * 512], m1[:], m2[:], OP.add)

                    # V (natural layout) for the 4 s-tiles of this chunk
                    for stl in range(4):
                        st = sc * 4 + stl
                        ps = qkv_ps.tile([128, 512], F32, tag="mm", bufs=2, name="psv")
                        for kt in range(NKT):
                            nc.tensor.matmul(
                                ps[:, 0:E], xta[:, kt * 512 + stl * 128: kt * 512 + (stl + 1) * 128],
                                wv_s[:, kt * E:(kt + 1) * E], start=(kt == 0), stop=(kt == NKT - 1))
                        if phase == 0:
                            continue
                        dst = vv_r[:, st, :, 0:HD]
                        nc.scalar.copy(dst, ps[:, 0:E].rearrange("p (h c) -> p h c", h=HPC, c=HD))

                # ================= attention + output projection =================
                # Per (q-chunk, head-pair): the two heads' K=64 score matmuls
                # sit in different PE row groups (partition bases 0/64) and
                # overlap in the array.  k-tiles are processed two at a time
                # so one exp covers a [128,1024] two-bank PSUM tile.  The
                # causal mask is ADDED into the scores by an identity matmul
                # of a 0/-400 constant (exp then yields exact-enough zeros).
                VW = HD + 1

                def emit_oproj(qc):
                    CW = min(512, D)
                    NPAIR = 2 if S >= 512 else 1
                    for st2 in range(4 // NPAIR):
                        oev = oevp.tile([128, NPAIR * D], F32, tag="oev", name="oev")
                        for half in range(NPAIR):
                            st = qc * 4 + st2 * NPAIR + half
                            for nch in range(D // CW):
                                op = mm_ps.tile([128, 512], F32, tag="mm", bufs=2, name="opps")
                                for g in range(G):
                                    nc.tensor.matmul(
                                        op[:, 0:CW], otn[:, g * S + st * 128:g * S + (st + 1) * 128],
                                        wo_s[:, g * D + nch * CW: g * D + (nch + 1) * CW],
                                        start=(g == 0), stop=(g == G - 1))
                                if nch % 2 == 0:
                                    nc.scalar.copy(
                                        oev[:, half * D + nch * CW: half * D + (nch + 1) * CW], op[:, 0:CW])
                                else:
                                    nc.vector.tensor_copy(
                                        oev[:, half * D + nch * CW: half * D + (nch + 1) * CW], op[:, 0:CW])
                        st0 = qc * 4 + st2 * NPAIR
                        dst = out[st0 * 128:(st0 + NPAIR) * 128, :].rearrange(
                            "(b p) d -> p b d", b=NPAIR)
                        nc.sync.dma_start(dst, oev[:].rearrange("p (b d) -> p b d", b=NPAIR))

                def attention_qc(qc):
                    for g in range(G):
                        h0, h1 = 2 * g, 2 * g + 1
                        otA = ot_ps.tile([128, 512], F32, tag="ot", name="otA")
                        otB = ot_ps.tile([128, 512], F32, tag="ot", name="otB")
                        nkt_q = 4 * qc + 4
                        for kt2 in range(0, nkt_q, 2):
                            kts = [kt2] + ([kt2 + 1] if kt2 + 1 < nkt_q else [])
                            sA = mm_ps.tile([128, 1024], F32, tag="sc2", bufs=2, name="sA")
                            sB = mm_ps.tile([128, 1024], F32, tag="sc2", bufs=2, name="sB")
                            for j, kt in enumerate(kts):
                                coff = max(0, 128 * kt - 512 * qc)
                                diag = kt >= 4 * qc
                                for hp, st in ((0, sA), (64, sB)):
                                    nc.tensor.matmul(
                                        st[:, j * 512 + coff: (j + 1) * 512],
                                        ktr[hp:hp + 64, g * S + kt * 128: g * S + (kt + 1) * 128],
                                        qtr[hp:hp + 64, g * S + qc * 512 + coff: g * S + (qc + 1) * 512],
                                        start=True, stop=not diag)
                                    if diag:
                                        nc.tensor.matmul(
                                            st[:, j * 512 + coff: j * 512 + coff + 128],
                                            idn_s[:], tri_s[:], start=False, stop=True)
                            c0 = max(0, 128 * kt2 - 512 * qc)
                            cend = len(kts) * 512
                            ptA = ptp.tile([128, 1024], F32R, tag="pt", name="ptA")
                            ptB = ptp.tile([128, 1024], F32R, tag="pt", name="ptB")
                            nc.scalar.activation(ptA[:, c0:cend], sA[:, c0:cend], AF.Exp, scale=0.125)
                            nc.scalar.activation(ptB[:, c0:cend], sB[:, c0:cend], AF.Exp, scale=0.125)
                            for j, kt in enumerate(kts):
                                coff = max(0, 128 * kt - 512 * qc)
                                for hh, pt, ot in ((h0, ptA, otA), (h1, ptB, otB)):
                                    nc.tensor.matmul(
                                        ot[0:VW, coff:512],
                                        vv[:, kt * HPC * VW + hh * VW: kt * HPC * VW + (hh + 1) * VW],
                                        pt[:, j * 512 + coff: (j + 1) * 512],
                                        start=(kt == 0), stop=(kt == nkt_q - 1))
                        # normalize by the denominator row: reciprocal of row
                        # 64, broadcast across 64 partitions via a K=1 matmul,
                        # then multiply out of PSUM.
                        if phase < 3:
                            continue
                        # batch both heads' normalize stages so the PE is
                        # visited once per pair and chain latencies overlap
                        rcA = rcp.tile([128, 512], F32R, tag="rcA", name="rcA")
                        rcB = rcp.tile([128, 512], F32R, tag="rcB", name="rcB")
                        nc.vector.reciprocal(rcA[HD:HD + 1, :], otA[HD:HD + 1, :])
                        nc.vector.reciprocal(rcB[HD:HD + 1, :], otB[HD:HD + 1, :])
                        rbA = mm_ps.tile([128, 512], F32, tag="mm", bufs=2, name="rbA")
                        rbB = mm_ps.tile([128, 512], F32, tag="mm", bufs=2, name="rbB")
                        nc.tensor.matmul(rbA[0:HD, :], on_s[HD:HD + 1, 0:HD],
                                         rcA[HD:HD + 1, :], start=True, stop=True)
                        nc.tensor.matmul(rbB[0:HD, :], on_s[HD:HD + 1, 0:HD],
                                         rcB[HD:HD + 1, :], start=True, stop=True)
                        nc.vector.tensor_copy(rcA[0:HD, :], rbA[0:HD, :])
                        nc.vector.tensor_copy(rcB[0:HD, :], rbB[0:HD, :])
                        for hp, ot, rc in ((0, otA, rcA), (64, otB, rcB)):
                            nc.vector.tensor_tensor(
                                otn[hp:hp + 64, g * S + qc * 512: g * S + (qc + 1) * 512],
                                ot[0:HD, :], rc[0:HD, :], OP.mult)



                # drive: interleave attention (and deferred O-proj) with the
                # QKV chunks -- attention for q-chunk sc needs only K/Q chunks
                # <= sc, so ACT's exp work overlaps the PE-dense projections.
                for sc in range(NSC):
                    qkv_chunk(sc)
                    if phase >= 2:
                        attention_qc(sc)
                    if phase >= 4 and sc > 0:
                        emit_oproj(sc - 1)
                if phase >= 4:
                    emit_oproj(NSC - 1)

            if use_loop:
                with tc.For_i(0, repeat, 1):
                    body()
            else:
                for _ in range(repeat):
                    body()

    _split_waits(nc)
    return nc


def _rope_tables(S):
    # interleaved dim order: within each 64-partition head block, partition
    # j=2i holds dim i (gets cos, -sin), j=2i+1 holds dim 32+i (cos, +sin)
    inv = 1.0 / (ROPE_BASE ** (np.arange(HALF, dtype=np.float64) / HALF))
    ang = np.arange(S, dtype=np.float64)[:, None] * inv[None, :]  # [S, HALF]
    cos, sin = np.cos(ang), np.sin(ang)
    j = np.arange(128) % HD
    freq = j // 2
    cs = cos[:, freq].T.astype(np.float32)                # [128, S]
    sgn = np.where(j % 2 == 0, -1.0, 1.0)
    sn = (sin[:, freq] * sgn[None, :]).T.astype(np.float32)
    return np.ascontiguousarray(cs), np.ascontiguousarray(sn)


def _tile_rows(a, nt):
    """[nt*128, C] -> [128, nt*C] with block kt at cols [kt*C, (kt+1)*C)."""
    n, c = a.shape
    assert n == nt * 128
    return np.ascontiguousarray(a.reshape(nt, 128, c).transpose(1, 0, 2).reshape(128, nt * c))


def _prep_x(x_b, D, S):
    """[S, D] -> [128, S*NKT]: col sc*(NKT*512) + kt*512 + s' = x[sc*512+s', kt*128+p]."""
    NKT, NSC = D // 128, S // 512
    return np.ascontiguousarray(
        x_b.reshape(NSC, 512, NKT, 128).transpose(3, 0, 2, 1).reshape(128, S * NKT))


def _mask_consts():
    # additive causal mask in [k, q] layout: 0 where k <= q, else -400
    # (-50 after the 1/8 softmax scale -> exp underflows to ~2e-22)
    tri = np.where(np.triu(np.ones((128, 128), dtype=bool)), 0.0, -400.0).astype(np.float32)
    idn = np.eye(128, dtype=np.float32)
    return tri, idn


def _interleave_perm(n_heads):
    """Permutation of head-dim rows: new row 64h+2i <- old 64h+i,
    new row 64h+2i+1 <- old 64h+32+i."""
    perm = np.empty(n_heads * HD, dtype=np.int64)
    for h in range(n_heads):
        base = h * HD
        for i in range(HALF):
            perm[base + 2 * i] = base + i
            perm[base + 2 * i + 1] = base + HALF + i
    return perm


_PROG_CACHE = {}


def kernel(x, Wq, Wk, Wv, Wo):
    B, S, D = x.shape
    H = 16
    HPC = 4                      # heads per core
    GROUPS = H // HPC            # 4 head-groups
    N_CORES = B * GROUPS

    x = np.asarray(x, dtype=np.float32)
    Wq, Wk, Wv, Wo = (np.asarray(w, dtype=np.float32) for w in (Wq, Wk, Wv, Wo))

    cs, sn = _rope_tables(S)
    tri, idn = _mask_consts()
    NKT = D // 128
    xTs = [_prep_x(x[b], D, S) for b in range(B)]

    perm = _interleave_perm(HPC)
    in_maps = []
    for c in range(N_CORES):
        b, hg = divmod(c, GROUPS)
        e0 = hg * HPC * HD
        e1 = e0 + HPC * HD
        in_maps.append({
            "xT": xTs[b],
            "wq": _tile_rows(Wq[e0:e1, :][perm].T, NKT),
            "wk": _tile_rows(Wk[e0:e1, :][perm].T, NKT),
            "wv": _tile_rows(Wv[e0:e1, :].T, NKT),
            "wo": _tile_rows(Wo[:, e0:e1].T, 2),
            "cs": cs, "sn": sn, "tri": tri, "idn": idn,
        })

    key = (S, D, HPC)
    if key not in _PROG_CACHE:
        _PROG_CACHE[key] = build_program(S, D, HPC)
    nc = _PROG_CACHE[key]
    res = run_bass_kernel_spmd(nc, in_maps, list(range(N_CORES)))

    out = np.zeros((B, S, D), dtype=np.float64)
    for c in range(N_CORES):
        b = c // GROUPS
        out[b] += res.results[c]["out"].astype(np.float64)
    return out.astype(np.float32)


if __name__ == "__main__":
    # mini self-test: one core, small S/D, against a numpy model
    S, D, HPC = 512, 256, 4
    rng = np.random.default_rng(0)
    x = rng.standard_normal((S, D)).astype(np.float32)
    bound = 1.0 / np.sqrt(D)
    Wq, Wk, Wv = (rng.uniform(-bound, bound, (HPC * HD, D)).astype(np.float32) for _ in range(3))
    Wo = rng.uniform(-bound, bound, (D, HPC * HD)).astype(np.float32)

    # numpy reference (same math as reference.py, restricted to HPC heads)
    q = (x @ Wq.T).reshape(S, HPC, HD).transpose(1, 0, 2)
    k = (x @ Wk.T).reshape(S, HPC, HD).transpose(1, 0, 2)
    v = (x @ Wv.T).reshape(S, HPC, HD).transpose(1, 0, 2)
    inv = 1.0 / (ROPE_BASE ** (np.arange(HALF) / HALF))
    ang = np.arange(S)[:, None] * inv[None, :]
    cosr, sinr = np.cos(ang), np.sin(ang)

    def rope(t):
        t1, t2 = t[..., :HALF], t[..., HALF:]
        return np.concatenate([t1 * cosr - t2 * sinr, t1 * sinr + t2 * cosr], -1)

    q, k = rope(q), rope(k)
    sc_ = np.einsum("hqd,hkd->hqk", q, k) / np.sqrt(HD)
    mask = np.tril(np.ones((S, S), dtype=bool))
    sc_ = np.where(mask, sc_, -np.inf)
    p = np.exp(sc_ - sc_.max(-1, keepdims=True))
    p /= p.sum(-1, keepdims=True)
    ref = np.einsum("hqk,hkd->hqd", p, v).transpose(1, 0, 2).reshape(S, HPC * HD) @ Wo.T

    cs, sn = _rope_tables(S)
    tri, idn = _mask_consts()
    perm = _interleave_perm(HPC)
    in_map = {
        "xT": _prep_x(x, D, S),
        "wq": _tile_rows(Wq[perm].T, D // 128),
        "wk": _tile_rows(Wk[perm].T, D // 128),
        "wv": _tile_rows(Wv.T, D // 128),
        "wo": _tile_rows(Wo.T, 2),
        "cs": cs, "sn": sn, "tri": tri, "idn": idn,
    }
    nc = build_program(S, D, HPC)
    res = run_bass_kernel_spmd(nc, [in_map], [0])
    got = res.results[0]["out"]
    err = np.abs(got - ref)
    rel = err.max() / np.abs(ref).max()
    rms = np.sqrt((err ** 2).mean()) / np.sqrt((ref ** 2).mean())
    print(f"mini: max abs err {err.max():.3e}  max rel {rel:.3e}  rms rel {rms:.3e}")
